# revision 2
# baseline (speedup 1.0000x reference)
"""Trainium2 Bass kernel for BinarizeLinear: y = x @ sign(W).T + bias.

Full-input contract: kernel(x=[65536,1024]f32, weight=[1024,1024]f32,
bias=[1024]f32) -> y=[65536,1024]f32.

Strategy (data-parallel, 8 NeuronCores):
  - Shard the batch dim of x 8 ways (8192 rows/core); replicate the
    binarized weight and bias (per the sharding hint).
  - Host precomputes sT = sign(W).T as bf16 [in_f, out_f] (+-1 exact in
    bf16) and casts x to bf16 (quantization ~1e-3 norm-relative, PSUM
    accumulation stays fp32).
  - Per core, the PE does ONLY matmuls: x is loaded straight into the
    transposed [in_f on partitions, batch free] layout by the DMA xbar
    transpose (16-bit dtype, DRAM->SBUF), so no PE transposes and no
    DVE psum evictions for x. 8 k-tile matmuls (K=128, N=512) accumulate
    in fp32 PSUM; DVE adds bias while evicting to bf16; DMA out.
  - bf16 matmul runs at 1 cycle/row: the PE matmul roofline is
    64 batch tiles x 16 matmuls x 512 rows ~= 187 us/pass; xbar-transpose
    x loads (~57 us) and bf16 y stores (~46 us) overlap underneath.
"""

from contextlib import ExitStack

import numpy as np

N_CORES = 8
B = 65536
IN_F = 1024
OUT_F = 1024
P = 128
B_SHARD = B // N_CORES  # 8192

_NC_CACHE = {}


def build_nc(
    b_shard=B_SHARD,
    repeat=1,
    hw_loop=0,
    chunk=8,
    xt_bufs=2,
    y_bufs=2,
    skip_mm=False,
    skip_xbar=False,
):
    """Build the per-core Bass module (SPMD: same program on all cores).

    hw_loop>0 wraps the main loop in a tc.For_i hardware loop running
    hw_loop times (same I/O each iteration); repeat>1 unrolls it;
    skip_mm/skip_xbar drop pipeline stages — benchmarking only.
    """
    import concourse.mybir as mybir
    import concourse.tile as tile
    from concourse import bacc

    f32 = mybir.dt.float32
    bf16 = mybir.dt.bfloat16
    KT = IN_F // P  # 8 k-tiles (contraction)
    NH = OUT_F // 512  # 2 psum halves
    CB = chunk  # batch tiles (128 rows) per chunk
    CR = CB * P  # rows per chunk
    NCH = b_shard // CR

    nc = bacc.Bacc("TRN2", target_bir_lowering=False, debug=False)
    x_d = nc.dram_tensor("x", [b_shard, IN_F], bf16, kind="ExternalInput")
    # wt = sign(W).T, host-precomputed bf16 [in_f, out_f]
    wt_d = nc.dram_tensor("wt", [IN_F, OUT_F], bf16, kind="ExternalInput")
    b_d = nc.dram_tensor("bias", [1, OUT_F], f32, kind="ExternalInput")
    y_d = nc.dram_tensor("y", [b_shard, OUT_F], bf16, kind="ExternalOutput")

    with tile.TileContext(nc) as tc, ExitStack() as ctx:
        const = ctx.enter_context(tc.tile_pool(name="const", bufs=1))
        sT_pool = ctx.enter_context(tc.tile_pool(name="sT", bufs=1))
        xT_pool = ctx.enter_context(tc.tile_pool(name="xT", bufs=xt_bufs))
        y_pool = ctx.enter_context(tc.tile_pool(name="yout", bufs=y_bufs))
        mm_psum = ctx.enter_context(tc.tile_pool(name="mmp", bufs=8, space="PSUM"))

        # ---- weights: sign(W).T already computed on host, straight DMA ----
        sT = [
            sT_pool.tile([P, OUT_F], bf16, tag=f"sT{ki}", name=f"sT{ki}")
            for ki in range(KT)
        ]
        for ki in range(KT):
            nc.sync.dma_start(sT[ki][:, :], wt_d.ap()[ki * P : (ki + 1) * P, :])

        # ---- bias: broadcast [1, OUT_F] -> [P, OUT_F] via a K=1 matmul ----
        bias_sb = const.tile([1, OUT_F], f32)
        nc.sync.dma_start(bias_sb[:, :], b_d.ap()[:, :])
        ones1 = const.tile([1, P], f32)
        nc.vector.memset(ones1[:, :], 1.0)
        bias_rep = const.tile([P, OUT_F], f32)
        for h in range(NH):
            bps = mm_psum.tile([P, 512], f32, tag="mm")
            nc.tensor.matmul(
                bps[:, :],
                ones1[:, :],
                bias_sb[:, h * 512 : (h + 1) * 512],
                start=True,
                stop=True,
            )
            nc.scalar.copy(bias_rep[:, h * 512 : (h + 1) * 512], bps[:, :])

        # ---- main loop over chunks of CB*128 batch rows ----
        loop_ctx = tc.For_i(0, hw_loop, 1) if hw_loop else None
        if loop_ctx is not None:
            loop_ctx.__enter__()
        for c in [t for _ in range(repeat) for t in range(NCH)]:
            # x chunk straight into transposed layout: xT[:, ki*CR + b] =
            # x[c*CR + b, ki*128 + p] via the DMA xbar (16x128 src tiles)
            xT = xT_pool.tile([P, KT * CR], bf16, tag="xT", name="xT")
            if not skip_xbar:
                for ki in range(KT):
                    nc.sync.dma_start(
                        xT[:, ki * CR : (ki + 1) * CR],
                        x_d.ap()[c * CR : (c + 1) * CR, ki * P : (ki + 1) * P],
                        transpose=True,
                    )
            y_sb = y_pool.tile([P, CB * OUT_F], bf16, tag="y", name="y_sb")
            for n in range(CB) if not skip_mm else []:
                for h in range(NH):
                    mm = mm_psum.tile([P, 512], f32, tag="mm")
                    for ki in range(KT):
                        nc.tensor.matmul(
                            mm[:, :],
                            xT[:, ki * CR + n * P : ki * CR + (n + 1) * P],
                            sT[ki][:, h * 512 : (h + 1) * 512],
                            start=(ki == 0),
                            stop=(ki == KT - 1),
                        )
                    nc.vector.tensor_add(
                        y_sb[:, n * OUT_F + h * 512 : n * OUT_F + (h + 1) * 512],
                        mm[:, :],
                        bias_rep[:, h * 512 : (h + 1) * 512],
                    )
            if skip_mm:
                nc.vector.tensor_copy(y_sb[:, : KT * CR], xT[:, :])
            # partition p holds batch row n*128+p of the chunk: per-partition
            # lines are CB contiguous 2KB rows
            nc.sync.dma_start(
                y_d.ap()[c * CR : (c + 1) * CR, :].rearrange("(n p) m -> p n m", p=P),
                y_sb[:, :].rearrange("p (n m) -> p n m", n=CB),
            )
        if loop_ctx is not None:
            loop_ctx.__exit__(None, None, None)

    nc.compile()
    return nc


def _get_nc(b_shard=B_SHARD):
    if b_shard not in _NC_CACHE:
        _NC_CACHE[b_shard] = build_nc(b_shard)
    return _NC_CACHE[b_shard]


def make_in_maps(x, weight, bias):
    import ml_dtypes

    bf16 = ml_dtypes.bfloat16
    x = np.asarray(x, dtype=np.float32)
    weight = np.asarray(weight, dtype=np.float32)
    # sign in f32 (exact {-1,0,+1}), transpose to [in_f, out_f], bf16 exact
    wt = np.ascontiguousarray(np.sign(weight).T.astype(bf16))
    bias = np.ascontiguousarray(np.asarray(bias, dtype=np.float32)).reshape(1, OUT_F)
    xb = x.astype(bf16)
    shard = x.shape[0] // N_CORES
    return [
        {
            "x": np.ascontiguousarray(xb[c * shard : (c + 1) * shard]),
            "wt": wt,
            "bias": bias,
        }
        for c in range(N_CORES)
    ], shard


def run(x, weight, bias, trace=False, **kwargs):
    """Run on 8 cores; returns (y_full_f32, BassKernelResults)."""
    from concourse.bass_utils import run_bass_kernel_spmd

    in_maps, shard = make_in_maps(x, weight, bias)
    nc = _get_nc(shard)
    res = run_bass_kernel_spmd(
        nc, in_maps, core_ids=list(range(N_CORES)), trace=trace, **kwargs
    )
    y = np.concatenate(
        [np.asarray(res.results[c]["y"], dtype=np.float32) for c in range(N_CORES)],
        axis=0,
    )
    return y, res


def kernel(x, weight, bias):
    y, _ = run(x, weight, bias)
    return np.asarray(y, dtype=np.float32)


# revision 7
# speedup vs baseline: 1.2502x; 1.2502x over previous
"""Trainium2 Bass kernel for BinarizeLinear: y = x @ sign(W).T + bias.

Full-input contract: kernel(x=[65536,1024]f32, weight=[1024,1024]f32,
bias=[1024]f32) -> y=[65536,1024]f32.

Strategy (data-parallel, 8 NeuronCores):
  - Shard the batch dim of x 8 ways (8192 rows/core); replicate the
    binarized weight and bias (per the sharding hint).
  - Host precomputes sT = sign(W).T as bf16 [in_f, out_f] (+-1 exact in
    bf16) and casts x to bf16 (quantization ~1e-3 norm-relative, PSUM
    accumulation stays fp32).
  - Per core, the PE does ONLY matmuls: x is loaded straight into the
    transposed [in_f on partitions, batch free] layout by the DMA xbar
    transpose (16-bit dtype, DRAM->SBUF), so no PE transposes and no
    DVE psum evictions for x. 8 k-tile matmuls (K=128, N=512) accumulate
    in fp32 PSUM; DVE adds bias while evicting to bf16; DMA out.
  - bf16 matmul runs at 1 cycle/row: the PE matmul roofline is
    64 batch tiles x 16 matmuls x 512 rows ~= 187 us/pass; xbar-transpose
    x loads (~57 us) and bf16 y stores (~46 us) overlap underneath.
"""

from contextlib import ExitStack

import numpy as np

N_CORES = 8
B = 65536
IN_F = 1024
OUT_F = 1024
P = 128
B_SHARD = B // N_CORES  # 8192

# x-path: "xbar" = DMA xbar-transpose loads from row-major x bf16;
# "host_T" = host ships x^T per core [IN_F, shard] bf16, plain DMAs.
X_MODE = "host_T"

_NC_CACHE = {}


def build_nc(
    b_shard=B_SHARD,
    repeat=1,
    hw_loop=0,
    chunk=8,
    xt_bufs=2,
    y_bufs=2,
    skip_mm=False,
    skip_xbar=False,
    x_mode=None,
):
    """Build the per-core Bass module (SPMD: same program on all cores).

    hw_loop>0 wraps the main loop in a tc.For_i hardware loop running
    hw_loop times (same I/O each iteration); repeat>1 unrolls it;
    skip_mm/skip_xbar drop pipeline stages — benchmarking only.
    """
    import concourse.mybir as mybir
    import concourse.tile as tile
    from concourse import bacc

    if x_mode is None:
        x_mode = X_MODE
    f32 = mybir.dt.float32
    bf16 = mybir.dt.bfloat16
    KT = IN_F // P  # 8 k-tiles (contraction)
    NH = OUT_F // 512  # 2 psum halves
    CB = chunk  # batch tiles (128 rows) per chunk
    CR = CB * P  # rows per chunk
    NCH = b_shard // CR

    nc = bacc.Bacc("TRN2", target_bir_lowering=False, debug=False)
    if x_mode == "host_T":
        x_d = nc.dram_tensor("x", [IN_F, b_shard], bf16, kind="ExternalInput")
    else:
        x_d = nc.dram_tensor("x", [b_shard, IN_F], bf16, kind="ExternalInput")
    # wt = sign(W).T, host-precomputed bf16 [in_f, out_f]
    wt_d = nc.dram_tensor("wt", [IN_F, OUT_F], bf16, kind="ExternalInput")
    b_d = nc.dram_tensor("bias", [1, OUT_F], f32, kind="ExternalInput")
    y_d = nc.dram_tensor("y", [b_shard, OUT_F], bf16, kind="ExternalOutput")

    with tile.TileContext(nc) as tc, ExitStack() as ctx:
        const = ctx.enter_context(tc.tile_pool(name="const", bufs=1))
        sT_pool = ctx.enter_context(tc.tile_pool(name="sT", bufs=1))
        xT_pool = ctx.enter_context(tc.tile_pool(name="xT", bufs=xt_bufs))
        y_pool = ctx.enter_context(tc.tile_pool(name="yout", bufs=y_bufs))
        mm_psum = ctx.enter_context(tc.tile_pool(name="mmp", bufs=8, space="PSUM"))

        # ---- weights: sign(W).T already computed on host, straight DMA ----
        sT = [
            sT_pool.tile([P, OUT_F], bf16, tag=f"sT{ki}", name=f"sT{ki}")
            for ki in range(KT)
        ]
        for ki in range(KT):
            nc.sync.dma_start(sT[ki][:, :], wt_d.ap()[ki * P : (ki + 1) * P, :])

        # ---- bias: broadcast [1, OUT_F] -> [P, OUT_F] via a K=1 matmul ----
        bias_sb = const.tile([1, OUT_F], f32)
        nc.sync.dma_start(bias_sb[:, :], b_d.ap()[:, :])
        ones1 = const.tile([1, P], f32)
        nc.vector.memset(ones1[:, :], 1.0)
        bias_rep = const.tile([P, OUT_F], f32)
        for h in range(NH):
            bps = mm_psum.tile([P, 512], f32, tag="mm")
            nc.tensor.matmul(
                bps[:, :],
                ones1[:, :],
                bias_sb[:, h * 512 : (h + 1) * 512],
                start=True,
                stop=True,
            )
            nc.scalar.copy(bias_rep[:, h * 512 : (h + 1) * 512], bps[:, :])

        # ---- main loop over chunks of CB*128 batch rows ----
        loop_ctx = tc.For_i(0, hw_loop, 1) if hw_loop else None
        if loop_ctx is not None:
            loop_ctx.__enter__()
        for c in [t for _ in range(repeat) for t in range(NCH)]:
            # x chunk straight into transposed layout: xT[:, ki*CR + b] =
            # x[c*CR + b, ki*128 + p] via the DMA xbar (16x128 src tiles)
            xT = xT_pool.tile([P, KT * CR], bf16, tag="xT", name="xT")
            if not skip_xbar:
                for ki in range(KT):
                    if x_mode == "host_T":
                        nc.sync.dma_start(
                            xT[:, ki * CR : (ki + 1) * CR],
                            x_d.ap()[ki * P : (ki + 1) * P, c * CR : (c + 1) * CR],
                        )
                    else:
                        nc.sync.dma_start(
                            xT[:, ki * CR : (ki + 1) * CR],
                            x_d.ap()[c * CR : (c + 1) * CR, ki * P : (ki + 1) * P],
                            transpose=True,
                        )
            y_sb = y_pool.tile([P, CB * OUT_F], bf16, tag="y", name="y_sb")
            for n in range(CB) if not skip_mm else []:
                for h in range(NH):
                    mm = mm_psum.tile([P, 512], f32, tag="mm")
                    for ki in range(KT):
                        nc.tensor.matmul(
                            mm[:, :],
                            xT[:, ki * CR + n * P : ki * CR + (n + 1) * P],
                            sT[ki][:, h * 512 : (h + 1) * 512],
                            start=(ki == 0),
                            stop=(ki == KT - 1),
                        )
                    nc.vector.tensor_add(
                        y_sb[:, n * OUT_F + h * 512 : n * OUT_F + (h + 1) * 512],
                        mm[:, :],
                        bias_rep[:, h * 512 : (h + 1) * 512],
                    )
            if skip_mm:
                nc.vector.tensor_copy(y_sb[:, : KT * CR], xT[:, :])
            # partition p holds batch row n*128+p of the chunk: per-partition
            # lines are CB contiguous 2KB rows
            nc.sync.dma_start(
                y_d.ap()[c * CR : (c + 1) * CR, :].rearrange("(n p) m -> p n m", p=P),
                y_sb[:, :].rearrange("p (n m) -> p n m", n=CB),
            )
        if loop_ctx is not None:
            loop_ctx.__exit__(None, None, None)

    nc.compile()
    return nc


def _get_nc(b_shard=B_SHARD):
    if b_shard not in _NC_CACHE:
        _NC_CACHE[b_shard] = build_nc(b_shard)
    return _NC_CACHE[b_shard]


def make_in_maps(x, weight, bias, x_mode=None):
    import ml_dtypes

    if x_mode is None:
        x_mode = X_MODE
    bf16 = ml_dtypes.bfloat16
    x = np.asarray(x, dtype=np.float32)
    weight = np.asarray(weight, dtype=np.float32)
    # sign in f32 (exact {-1,0,+1}), transpose to [in_f, out_f], bf16 exact
    wt = np.ascontiguousarray(np.sign(weight).T.astype(bf16))
    bias = np.ascontiguousarray(np.asarray(bias, dtype=np.float32)).reshape(1, OUT_F)
    shard = x.shape[0] // N_CORES
    if x_mode == "host_T":
        # per-core x^T [IN_F, shard]: one transpose of the full bf16 array,
        # then contiguous column-block slices
        xT = np.ascontiguousarray(x.astype(bf16).T)
        xs = [
            np.ascontiguousarray(xT[:, c * shard : (c + 1) * shard])
            for c in range(N_CORES)
        ]
    else:
        xb = x.astype(bf16)
        xs = [
            np.ascontiguousarray(xb[c * shard : (c + 1) * shard])
            for c in range(N_CORES)
        ]
    return [
        {"x": xs[c], "wt": wt, "bias": bias} for c in range(N_CORES)
    ], shard


def run(x, weight, bias, trace=False, **kwargs):
    """Run on 8 cores; returns (y_full_f32, BassKernelResults)."""
    from concourse.bass_utils import run_bass_kernel_spmd

    in_maps, shard = make_in_maps(x, weight, bias)
    nc = _get_nc(shard)
    res = run_bass_kernel_spmd(
        nc, in_maps, core_ids=list(range(N_CORES)), trace=trace, **kwargs
    )
    y = np.concatenate(
        [np.asarray(res.results[c]["y"], dtype=np.float32) for c in range(N_CORES)],
        axis=0,
    )
    return y, res


def kernel(x, weight, bias):
    y, _ = run(x, weight, bias)
    return np.asarray(y, dtype=np.float32)
